# revision 31
# baseline (speedup 1.0000x reference)
"""Trainium2 Bass kernel for multi-head causal attention with RoPE.

Problem: x[2,2048,1024] @ {qw,kw,vw}[1024,1024] -> 16-head causal attention
with interleaved RoPE on Q,K -> @ ow[1024,1024].

Sharding (8 cores): core c handles batch b=c//4, head-group g=c%4 (4 heads).
qw/kw/vw column-sharded, ow row-sharded (Megatron attention parallelism);
the host sums the 4 partial outputs per batch.

Device-side strategy (per core), v3 — fused single-pass pipeline:
- Q^T/K^T computed DIRECTLY transposed: stationary = weight chunk
  [128k x 128d], moving = x^T chunk [128k x 512s], PSUM accumulates over
  the 8 model-dim chunks.  No PE transposes anywhere in the kernel.
- Pair-tile row order [h0e h0o h1e h1o] (evens|odds deinterleaved per
  head via a host-side weight-column permutation, which cancels in the
  Q.K dot products).  RoPE rot = P*CS + swap32(P)*SN:
    * scalar engine evicts PSUM -> pse (bf16),
    * one DVE op computes t12 = pse * [CS | SN'] (stride-0 broadcast
      read of pse, SN' pre-swapped by the host),
    * DMA performs the 32-row swap of the SN' half (off-engine),
    * GpSimd adds the halves into the Q^T/K^T tile.
- S^T per k-block = two concurrent K=64 matmuls (heads at partition
  bases 0/64 -> distinct PE row groups, distinct PSUM halves); one
  scalar-engine Exp evicts both heads (scale=1/8; scores are small so
  no max subtraction is needed).
- PV: stationary = [ones*64 | V_h] (M=128): PSUM rows 0..63 all
  accumulate the softmax denominator, so one reciprocal_approx_fast
  reads rows 0:64 (base partition 0 — required for custom-DVE PSUM
  reads) and lands the broadcast 1/den straight in SBUF.  A
  scalar_tensor_tensor then multiplies rows 64:128 by it on the way
  out to the merged^T tile (no transposes, no extra copies).
- Out-projection: stationary = merged^T [128d x 128q], moving = ow rows
  [128 x 512]; [128,1024] fp32 partials DMA'd per q-block.
- Phase fusion: the attention inner loop is exp(ACT)-bound, so V
  projections, the second pair's Q/K projections and the
  out-projections are interleaved into the attention k-loops as PE
  filler.  DMA prologue ordered so the first matmul starts after
  ~1.5MB of traffic.
"""

import sys

sys.path.insert(0, "/opt/trn_rl_repo")

import numpy as np
import ml_dtypes

BF = ml_dtypes.bfloat16
from contextlib import ExitStack

import concourse.bass as bass
import concourse.bacc as bacc
import concourse.tile as tile
from concourse import mybir
from concourse.bass_utils import run_bass_kernel_spmd

F32 = mybir.dt.float32
BF16 = mybir.dt.bfloat16
FP8 = mybir.dt.float8e4
F8 = ml_dtypes.float8_e4m3fn
DR = mybir.MatmulPerfMode.DoubleRow
W_SCALE = 256.0  # wq/wk pre-scale to keep fp8 weights in the normal range
Exp = mybir.ActivationFunctionType.Exp
MULT = mybir.AluOpType.mult

D_MODEL, N_HEADS, BATCH, SEQ = 1024, 16, 2, 2048
HEAD_DIM = 64
N_CORES = 8
GH = 4  # heads per core
GD = GH * HEAD_DIM  # 256 cols per core
NB = SEQ // 128  # 16 s-blocks
NQC = SEQ // 512  # 4 q-chunks
NC_CHUNKS = 8  # model-dim chunks of 128


def build_program(phases=(1, 2, 3)):
    nc = bacc.Bacc("TRN2", target_bir_lowering=False)

    xT = nc.declare_dram_parameter("xT", [128, NC_CHUNKS * SEQ], BF16, isOutput=False)
    xT8 = nc.declare_dram_parameter("xT8", [128, NC_CHUNKS * SEQ], FP8, isOutput=False)
    wq = nc.declare_dram_parameter("wq", [128, NC_CHUNKS * GD], FP8, isOutput=False)
    wk = nc.declare_dram_parameter("wk", [128, NC_CHUNKS * GD], FP8, isOutput=False)
    wv = nc.declare_dram_parameter("wv", [128, NC_CHUNKS * GD], BF16, isOutput=False)
    wo = nc.declare_dram_parameter("wo", [128, 2 * D_MODEL], BF16, isOutput=False)
    cssn = nc.declare_dram_parameter("cssn", [128, 2 * SEQ], BF16, isOutput=False)
    tri = nc.declare_dram_parameter("tri", [128, 128], BF16, isOutput=False)
    out = nc.declare_dram_parameter("out", [SEQ, D_MODEL], BF16, isOutput=True)

    with tile.TileContext(nc) as tc, ExitStack() as ctx:
        res = ctx.enter_context(tc.tile_pool(name="res", bufs=1))
        # SBUF scratch pools
        sp_rope = ctx.enter_context(tc.tile_pool(name="rope", bufs=3))
        sp_wt = ctx.enter_context(tc.tile_pool(name="wt", bufs=4))
        sp_rbc = ctx.enter_context(tc.tile_pool(name="rbc", bufs=2))
        sp_ob = ctx.enter_context(tc.tile_pool(name="ob", bufs=3))
        # PSUM pools: 4 + 2 + 2 = 8 banks exactly
        pp_s = ctx.enter_context(tc.tile_pool(name="ps", bufs=2, space="PSUM"))
        pp_pv = ctx.enter_context(tc.tile_pool(name="ppv", bufs=1, space="PSUM"))
        pp_sc = ctx.enter_context(tc.tile_pool(name="psc", bufs=2, space="PSUM"))

        # resident tensors
        xt_sb = res.tile([128, NC_CHUNKS * SEQ], BF16, tag="xt")
        xt8_sb = res.tile([128, NC_CHUNKS * SEQ], FP8, tag="xt8")
        wq_sb = res.tile([128, NC_CHUNKS * GD], FP8, tag="wq")
        wk_sb = res.tile([128, NC_CHUNKS * GD], FP8, tag="wk")
        wv_sb = res.tile([128, NC_CHUNKS * GD], BF16, tag="wv")
        wo_sb = res.tile([128, 2 * D_MODEL], BF16, tag="wo")
        cssn_sb = res.tile([128, 2 * SEQ], BF16, tag="cssn")
        tri_sb = res.tile([128, 128], BF16, tag="tri")
        qt_sb = [res.tile([128, SEQ], BF16, tag=f"qt{p}", name=f"qt{p}") for p in range(2)]
        kt_sb = [res.tile([128, SEQ], BF16, tag=f"kt{p}", name=f"kt{p}") for p in range(2)]
        vaug_sb = res.tile([128, NB * GH * 128], BF16, tag="vaug")
        mt_sb = [res.tile([128, SEQ], BF16, tag=f"mt{p}", name=f"mt{p}") for p in range(2)]

        xt_v = xt_sb[:].rearrange("p (t c s) -> p t c s", t=NQC, c=NC_CHUNKS)
        xt8_v = xt8_sb[:].rearrange("p (t c j s) -> p t c j s", t=NQC, c=4, j=2)
        wq_v = wq_sb[:].rearrange("p (c j h m) -> p c j h m", c=4, j=2, h=2)
        wk_v = wk_sb[:].rearrange("p (c j h m) -> p c j h m", c=4, j=2, h=2)
        wv_v = wv_sb[:].rearrange("p (c d) -> p c d", c=NC_CHUNKS)
        wo_v = wo_sb[:].rearrange("p (h d) -> p h d", h=2)
        cssn_v = cssn_sb[:].rearrange("p (j s) -> p j s", j=2)
        vaug_v = vaug_sb[:].rearrange("p (n h e) -> p n h e", n=NB, h=GH)
        xT_v = xT[:].rearrange("p (t c s) -> p t c s", t=NQC, c=NC_CHUNKS)
        xT8_v = xT8[:].rearrange("p (t c j s) -> p t c j s", t=NQC, c=4, j=2)

        # ---- prologue DMAs (ordered so first matmuls can start early;
        # sync + scalar are the two hardware DGE queues) ----
        nc.gpsimd.dma_start(wq_sb[:], wq[:])
        nc.gpsimd.dma_start(wk_sb[:], wk[:])
        nc.gpsimd.dma_start(xt8_v[:, 0], xT8_v[:, 0])
        nc.sync.dma_start(xt_v[:, 0, 0:4], xT_v[:, 0, 0:4])
        nc.sync.dma_start(xt_v[:, 0, 4:8], xT_v[:, 0, 4:8])
        nc.gpsimd.dma_start(cssn_sb[:], cssn[:])
        nc.gpsimd.dma_start(wv_sb[:], wv[:])
        nc.gpsimd.dma_start(tri_sb[:], tri[:])
        # ones half of vaug (V evictions write cols 64-127 of each
        # 128-block; cols 0-63 = 1.0 -> PV rows 0:64 = denominator)
        nc.vector.memset(vaug_v[:, :, :, 0:64], 1.0)

        def xt_dma_items(sc):
            items = [
                lambda c=c: nc.sync.dma_start(xt8_v[:, sc, c], xT8_v[:, sc, c])
                for c in range(4)
            ]
            items += [
                lambda c=c: nc.sync.dma_start(xt_v[:, sc, c], xT_v[:, sc, c])
                for c in range(NC_CHUNKS)
            ]
            return items

        # warm the exp table load
        with tc.tile_pool(name="warm", bufs=1) as wpool:
            scratch = wpool.tile([128, 1], F32)
            nc.vector.memset(scratch[:], 0.0)
            nc.scalar.activation(scratch[:], scratch[:], Exp)

        # ---------------- emission helpers --------------------------------
        def emit_qk_proj(pair, kind, sc):
            """Q^T/K^T projection + RoPE for one pair-tile, one 512-s-chunk."""
            wsb = wq_v if kind == "q" else wk_v
            dst = qt_sb[pair] if kind == "q" else kt_sb[pair]
            ssl = slice(sc * 512, (sc + 1) * 512)
            ps = pp_sc.tile([128, 512], F32, tag="sc", name="psqk")
            for c in range(4):  # fp8 DoubleRow: two 128-chunks per matmul
                nc.tensor.matmul(
                    ps[:],
                    wsb[:, c, :, pair, :],
                    xt8_v[:, sc, c],
                    start=(c == 0),
                    stop=(c == 3),
                    perf_mode=DR,
                )
            # RoPE: rot = P*CS + swap32(P)*SN
            pse = sp_rope.tile([128, 512], BF16, tag="pse", name="pse")
            if kind == "q":
                nc.scalar.copy(pse[:], ps[:])
            else:
                nc.vector.tensor_copy(pse[:], ps[:])
            t12 = sp_rope.tile([128, 1024], BF16, tag="t12", name="t12")
            t12_v = t12[:].rearrange("p (j s) -> p j s", j=2)
            nc.vector.tensor_tensor(
                t12_v,
                pse[:].unsqueeze(1).broadcast_to([128, 2, 512]),
                cssn_v[:, :, ssl],
                op=MULT,
            )
            t2s = sp_rope.tile([128, 512], BF16, tag="t2s", name="t2s")
            for g in range(4):
                gs = slice(g * 32, g * 32 + 32)
                gd = slice((g ^ 1) * 32, (g ^ 1) * 32 + 32)
                nc.sync.dma_start(t2s[gd, :], t12_v[gs, 1, :])
            nc.gpsimd.tensor_add(dst[:, ssl], t12_v[:, 0, :], t2s[:])

        def emit_v_proj(sb):
            """V projection for one 128-row s-block into vaug."""
            ps = pp_sc.tile([128, 512], F32, tag="sc", name="psv")
            for c in range(NC_CHUNKS):
                nc.tensor.matmul(
                    ps[:, 0:GD],
                    xt_v[:, sb // 4, c, (sb % 4) * 128 : (sb % 4 + 1) * 128],
                    wv_v[:, c, :],
                    start=(c == 0),
                    stop=(c == NC_CHUNKS - 1),
                )
            nc.vector.tensor_copy(
                vaug_v[:, sb, :, 64:128],
                ps[:, 0:GD].rearrange("p (h d) -> p h d", h=GH),
            )

        def emit_outproj(qb):
            """Output projection for one 128-row q-block."""
            ob = sp_ob.tile([128, D_MODEL], BF16, tag="ob", name="ob")
            for oc in range(2):
                ps = pp_sc.tile([128, 512], F32, tag="sc", name="pso")
                for hp in range(2):
                    nc.tensor.matmul(
                        ps[:],
                        mt_sb[hp][:, qb * 128 : (qb + 1) * 128],
                        wo_v[:, hp, oc * 512 : (oc + 1) * 512],
                        start=(hp == 0),
                        stop=(hp == 1),
                    )
                nc.vector.tensor_copy(ob[:, oc * 512 : (oc + 1) * 512], ps[:])
            eng = nc.sync
            eng.dma_start(out[qb * 128 : (qb + 1) * 128, :], ob[:])

        def emit_attn(pair, qc, filler):
            """Causal attention for one pair, one 512-wide q-chunk."""
            nk = 4 * qc + 4
            n_inj = 0
            pv = pp_pv.tile([128, 1024], F32, tag="pv", name="pv")
            prev = None
            for ki in range(nk):
                j = ki - 4 * qc
                diag = j >= 0
                off = j * 128 if diag else 0
                n = 512 - off
                ps_s = pp_s.tile([128, 1024], F32, tag="s", name="pss")
                wt = sp_wt.tile([128, 1024], BF16, tag="wt", name="wt")
                # S^T: 2 concurrent K=64 matmuls (heads at partition bases
                # 0/64 -> distinct PE row groups, distinct PSUM halves)
                for e in range(2):
                    rows = slice(e * 64, e * 64 + 64)
                    nc.tensor.matmul(
                        ps_s[:, e * 512 : e * 512 + n],
                        kt_sb[pair][rows, ki * 128 : (ki + 1) * 128],
                        qt_sb[pair][rows, qc * 512 + off : (qc + 1) * 512],
                        start=True,
                        stop=True,
                    )
                ps_v = ps_s[:].rearrange("p (e q) -> p e q", e=2)
                wt_v = wt[:].rearrange("p (e q) -> p e q", e=2)
                nc.scalar.activation(
                    wt_v[:, :, 0:n],
                    ps_v[:, :, 0:n],
                    Exp,
                    scale=0.125 / (W_SCALE * W_SCALE),
                )
                if diag:
                    nc.vector.tensor_tensor(
                        wt_v[:, :, 0:128],
                        wt_v[:, :, 0:128],
                        tri_sb[:].unsqueeze(1).broadcast_to([128, 2, 128]),
                        op=MULT,
                    )
                cur = (ki, off, n, wt)
                if prev is not None:
                    pki, poff, pn, pwt = prev
                    for e in range(2):
                        nc.tensor.matmul(
                            pv[:, e * 512 + poff : e * 512 + 512],
                            vaug_v[:, pki, 2 * pair + e, :],
                            pwt[:, e * 512 : e * 512 + pn],
                            start=(pki == 0),
                            stop=False,
                        )
                prev = cur
                # drain filler work proportionally (front-loaded by one ki)
                target = min(len(filler), -(-((ki + 2) * len(filler)) // nk))
                while n_inj < target:
                    filler[n_inj]()
                    n_inj += 1
            pki, poff, pn, pwt = prev
            for e in range(2):
                nc.tensor.matmul(
                    pv[:, e * 512 + poff : e * 512 + 512],
                    vaug_v[:, pki, 2 * pair + e, :],
                    pwt[:, e * 512 : e * 512 + pn],
                    start=(pki == 0),
                    stop=True,
                )
            # tail: rows 0:64 of each pv half hold the softmax denominator
            # (replicated; recip_approx_fast needs base partition 0 for PSUM
            # inputs), rows 64:128 the raw attention.  One recip lands the
            # broadcast 1/den in SBUF, one fused multiply per head evicts
            # normalized merged^T.
            rbc = sp_rbc.tile([64, 1024], F32, tag="rbc", name="rbc")
            nc.vector.reciprocal_approx_fast(rbc[:], pv[0:64, :])
            for e in range(2):
                nc.vector.scalar_tensor_tensor(
                    mt_sb[pair][e * 64 : (e + 1) * 64, qc * 512 : (qc + 1) * 512],
                    pv[64:128, e * 512 : (e + 1) * 512],
                    1.0,
                    rbc[0:64, e * 512 : (e + 1) * 512],
                    op0=MULT,
                    op1=MULT,
                )

        # ---------------- fused schedule -----------------------------------
        if 1 in phases:
            # prologue: pair0 QK proj for sc0 + first V blocks
            emit_qk_proj(0, "q", 0)
            emit_qk_proj(0, "k", 0)
            for sb in range(4):
                emit_v_proj(sb)

            if 2 in phases:
                # stage B: pair0 attention, filler = rest of projections
                fillers = {
                    0: xt_dma_items(1)
                    + [lambda: emit_qk_proj(0, "q", 1)]
                    + [lambda: emit_qk_proj(0, "k", 1)]
                    + [lambda sb=sb: emit_v_proj(sb) for sb in (4, 5)],
                    1: xt_dma_items(2)
                    + [lambda: nc.sync.dma_start(wo_sb[:], wo[:])]
                    + [lambda: emit_qk_proj(0, "q", 2)]
                    + [lambda: emit_qk_proj(0, "k", 2)]
                    + [lambda sb=sb: emit_v_proj(sb) for sb in (6, 7, 8, 9)],
                    2: xt_dma_items(3)
                    + [lambda: emit_qk_proj(0, "q", 3)]
                    + [lambda: emit_qk_proj(0, "k", 3)]
                    + [lambda sb=sb: emit_v_proj(sb) for sb in (10, 11, 12, 13, 14, 15)],
                    3: [
                        lambda kind=kind, sc=sc: emit_qk_proj(1, kind, sc)
                        for sc in range(NQC)
                        for kind in ("q", "k")
                    ],
                }
                for qc in range(NQC):
                    emit_attn(0, qc, fillers[qc])
                # stage C: pair1 attention, filler = out-projections
                for qc in range(NQC):
                    f = (
                        [lambda qb=qb: emit_outproj(qb) for qb in range(4 * (qc - 1), 4 * qc)]
                        if (qc > 0 and 3 in phases)
                        else []
                    )
                    emit_attn(1, qc, f)
                if 3 in phases:
                    for qb in range(12, 16):
                        emit_outproj(qb)

    nc.compile()
    return nc


_NC = None


def _host_tables():
    # RoPE tables in transposed layout [128 rows, SEQ]; pair-tile rows are
    # [h0 evens, h0 odds, h1 evens, h1 odds] (32 rows each)
    inv_freq = 1.0 / (
        10000.0 ** (np.arange(0, HEAD_DIM, 2, dtype=np.float32) / HEAD_DIM)
    )
    pos = np.arange(SEQ, dtype=np.float32)
    freq = inv_freq[:, None] * pos[None, :]  # [32, SEQ]
    cospat = np.cos(freq).astype(np.float32)
    sinpat = np.sin(freq).astype(np.float32)
    CS = np.tile(cospat, (4, 1))  # [128, SEQ]
    SN = np.tile(np.concatenate([-sinpat, sinpat], axis=0), (2, 1))
    tri = np.triu(np.ones((128, 128), dtype=np.float32))  # keep k<=q
    # within-shard column permutation: per pair-tile [h0e, h0o, h1e, h1o]
    perm = []
    for p in range(2):
        for e in range(2):  # head within pair
            for par in range(2):  # 0: evens, 1: odds
                base = (2 * p + e) * 64
                perm.extend(base + 2 * np.arange(32) + par)
    perm = np.array(perm)
    return CS, SN, tri, perm


def _blk_w(w):
    # [C*128, D] -> [128, C*D] with chunk-major free dim
    c = w.shape[0] // 128
    return np.ascontiguousarray(
        w.reshape(c, 128, w.shape[1]).transpose(1, 0, 2).reshape(128, -1)
    ).astype(BF)


def _blk_w8(w):
    # [1024, 256] -> [128, (c2=4, j=2, pair=2, 128)] fp8, pre-scaled
    v = (w * W_SCALE).reshape(4, 2, 128, 2, 128).transpose(2, 0, 1, 3, 4)
    return np.ascontiguousarray(v.reshape(128, -1)).astype(F8)


def _blk_x8(xb):
    # x[b] [SEQ, 1024] -> xT -> [128, (sc=4, c2=4, j=2, 512)] fp8
    xT = xb.T
    v = xT.reshape(4, 2, 128, NQC, 512).transpose(2, 3, 0, 1, 4)
    return np.ascontiguousarray(v.reshape(128, -1)).astype(F8)


def _blk_x(xb):
    # x[b] [SEQ, D_MODEL] -> xT [1024, 2048] -> [128, (sc=4, c=8, 512)]
    xT = xb.T
    v = xT.reshape(NC_CHUNKS, 128, NQC, 512).transpose(1, 2, 0, 3)
    return np.ascontiguousarray(v.reshape(128, -1)).astype(BF)


def _in_maps(x, qw, kw, vw, ow):
    CS, SN, tri, perm = _host_tables()
    # SN' = SN with 32-row groups swapped, interleaved as [CS | SN']
    SNp = SN.reshape(4, 32, -1)[[1, 0, 3, 2]].reshape(128, -1)
    cssn = np.stack([CS, SNp], axis=1).reshape(128, 2 * SEQ)
    maps = []
    for c in range(N_CORES):
        b, g = c // GH, c % GH
        sl = slice(g * GD, (g + 1) * GD)
        maps.append(
            dict(
                xT=_blk_x(x[b]),
                xT8=_blk_x8(x[b]),
                wq=_blk_w8(qw[:, sl][:, perm]),
                wk=_blk_w8(kw[:, sl][:, perm]),
                wv=_blk_w(vw[:, sl]),
                wo=_blk_w(ow[sl, :]),
                cssn=cssn.astype(BF),
                tri=tri.astype(BF),
            )
        )
    return maps


def _run(x, qw, kw, vw, ow, trace=False):
    global _NC
    if _NC is None:
        _NC = build_program()
    maps = _in_maps(
        np.asarray(x, dtype=np.float32),
        np.asarray(qw, dtype=np.float32),
        np.asarray(kw, dtype=np.float32),
        np.asarray(vw, dtype=np.float32),
        np.asarray(ow, dtype=np.float32),
    )
    br = run_bass_kernel_spmd(_NC, maps, list(range(N_CORES)), trace=trace)
    out = np.zeros((BATCH, SEQ, D_MODEL), dtype=np.float32)
    for c in range(N_CORES):
        out[c // GH] += br.results[c]["out"].astype(np.float32)
    return out, br


def kernel(x, qw, kw, vw, ow):
    out, _ = _run(x, qw, kw, vw, ow)
    return out
